# revision 1
# baseline (speedup 1.0000x reference)
"""Multi-head attention (QKV proj + RoPE + softmax attention) on 8 Trainium2
NeuronCores, tensor-parallel over heads (2 heads per core).

Contract: kernel(**inputs) takes the FULL unsharded inputs (numpy/jax arrays,
shapes hardcoded below) and returns the FULL [B, S, H] output.
"""

from contextlib import ExitStack

import numpy as np

B, S, H = 2, 2048, 2048
NH, D = 16, 128
ROPE_BASE = 10000.0
NCORES = 8
HPC = NH // NCORES          # heads per core
CH = HPC * D                # output channels per core
BS = B * S                  # flattened tokens
KT = H // 128               # contraction k-tiles
NCH = BS // 512             # 512-wide token chunks
SKT = S // 128              # score k-tiles per sequence
SQC = S // 512              # query chunks per sequence

LAST_RESULT = None          # BassKernelResults of the most recent run (for test.py)


def _build_nc(with_bias):
    import concourse.mybir as mybir
    import concourse.tile as tile
    from concourse import bacc
    from concourse.masks import make_identity

    F32 = mybir.dt.float32
    F32R = mybir.dt.float32r
    AF = mybir.ActivationFunctionType
    ALU = mybir.AluOpType
    ISCALE = float(1.0 / np.sqrt(D))

    nc = bacc.Bacc("TRN2", debug=False, enable_partition_id=False)

    hsT_d = nc.dram_tensor("hsT", [H, BS], F32R, kind="ExternalInput").ap()
    wT_d = {
        p: nc.dram_tensor(f"w{p}T", [H, CH], F32R, kind="ExternalInput").ap()
        for p in "qkv"
    }
    b_d = {
        p: nc.dram_tensor(f"b{p}", [1, CH], F32R, kind="ExternalInput").ap()
        for p in "qkv"
    }
    cos_d = nc.dram_tensor("cosT", [D, S], F32, kind="ExternalInput").ap()
    sin_d = nc.dram_tensor("sinT", [D, S], F32, kind="ExternalInput").ap()
    out_d = nc.dram_tensor("out", [BS, CH], F32, kind="ExternalOutput").ap()

    with tile.TileContext(nc) as tc, ExitStack() as ctx:
        # ---- persistent state (lives across both phases) ----
        persist = ctx.enter_context(tc.tile_pool(name="persist", bufs=1))
        qT = [persist.tile([128, BS], F32R, tag=f"qT{m}", name=f"qT{m}") for m in range(HPC)]
        kTt = [persist.tile([128, BS], F32R, tag=f"kT{m}", name=f"kT{m}") for m in range(HPC)]
        vN = [persist.tile([128, BS // 128, D], F32R, tag=f"v{m}", name=f"vn{m}") for m in range(HPC)]

        consts = ctx.enter_context(tc.tile_pool(name="consts", bufs=1))
        ident = consts.tile([128, 128], F32, tag="ident")
        make_identity(nc, ident)
        ones_c = consts.tile([128, 32], F32, tag="ones_c")
        nc.vector.memset(ones_c, 1.0)
        if with_bias:
            ones_row = consts.tile([1, 512], F32, tag="ones_row")
            nc.vector.memset(ones_row, 1.0)
            b_sb = {}
            for p in "qkv":
                b_sb[p] = consts.tile([1, CH], F32R, tag=f"b{p}", name=f"b{p}sb")
                nc.sync.dma_start(b_sb[p], b_d[p])

        # ================= Phase 1: QKV projections + RoPE =================
        with (
            tc.tile_pool(name="wpool", bufs=1) as wpool,
            tc.tile_pool(name="tabs", bufs=1) as tabs,
            tc.tile_pool(name="hstp", bufs=8) as hstp,
            tc.tile_pool(name="p1ps", bufs=1, space="PSUM") as p1ps,
            tc.tile_pool(name="vtrp", bufs=2, space="PSUM") as vtrp,
            tc.tile_pool(name="ropet", bufs=3) as ropet,
        ):
            w_sb = {}
            for p in "qkv":
                w_sb[p] = wpool.tile([128, KT, CH], F32R, tag=f"w{p}", name=f"w{p}sb")
            # per-k interleaved weight loads so the first matmuls start early
            w_r = {p: wT_d[p].rearrange("(k p) c -> p k c", p=128) for p in "qkv"}
            for k in range(KT):
                for p in "qkv":
                    nc.sync.dma_start(w_sb[p][:, k, :], w_r[p][:, k, :])
            cos_sb = tabs.tile([D, S], F32, tag="cos")
            sin_sb = tabs.tile([D, S], F32, tag="sin")
            nc.sync.dma_start(cos_sb, cos_d)
            nc.sync.dma_start(sin_sb, sin_d)

            hsT_r = hsT_d.rearrange("(k p) t -> p k t", p=128)

            for n in range(NCH):
                tok = slice(n * 512, (n + 1) * 512)
                pos = slice((n % SQC) * 512, (n % SQC + 1) * 512)
                hs_t = [hstp.tile([128, 512], F32R, tag="hs", name=f"hs{k}") for k in range(KT)]
                for k in range(KT):
                    nc.sync.dma_start(hs_t[k], hsT_r[:, k, tok])

                prj_ps = {
                    (p, m): p1ps.tile([128, 512], F32, tag=f"{p}{m}", name=f"ps{p}{m}")
                    for p in "qkv"
                    for m in range(HPC)
                }
                for k in range(KT):
                    for p in "qkv":
                        for m in range(HPC):
                            nc.tensor.matmul(
                                prj_ps[(p, m)],
                                w_sb[p][:, k, m * 128:(m + 1) * 128],
                                hs_t[k],
                                start=(k == 0),
                                stop=(k == KT - 1) and not with_bias,
                            )
                if with_bias:
                    for p in "qkv":
                        for m in range(HPC):
                            nc.tensor.matmul(
                                prj_ps[(p, m)],
                                b_sb[p][:, m * 128:(m + 1) * 128],
                                ones_row.bitcast(F32R),
                                start=False,
                                stop=True,
                            )

                # RoPE on q/k (3 psum-reading ops then the final add)
                for p, dst in (("q", qT), ("k", kTt)):
                    for m in range(HPC):
                        ps = prj_ps[(p, m)]
                        t1 = ropet.tile([128, 512], F32, tag="t1")
                        nc.vector.tensor_tensor(t1, ps, cos_sb[:, pos], op=ALU.mult)
                        t2 = ropet.tile([128, 512], F32, tag="t2")
                        nc.vector.tensor_tensor(
                            t2[0:64], ps[64:128], sin_sb[64:128, pos], op=ALU.mult
                        )
                        nc.vector.tensor_tensor(
                            t2[64:128], ps[0:64], sin_sb[0:64, pos], op=ALU.mult
                        )
                        nc.vector.tensor_tensor(dst[m][:, tok], t1, t2, op=ALU.add)

                # v: copy psum -> sbuf, PE-transpose to natural [S, d] layout
                for m in range(HPC):
                    vt_sb = ropet.tile([128, 512], F32, tag="vt")
                    nc.vector.tensor_copy(vt_sb, prj_ps[("v", m)])
                    for blk in range(4):
                        vtr_ps = vtrp.tile([128, 128], F32, tag="vtr")
                        nc.tensor.transpose(
                            vtr_ps, vt_sb[:, blk * 128:(blk + 1) * 128], ident
                        )
                        nc.vector.tensor_copy(vN[m][:, n * 4 + blk, :], vtr_ps)

        # ================= Phase 2: attention =================
        with (
            tc.tile_pool(name="epool", bufs=20) as epool,
            tc.tile_pool(name="opool", bufs=4) as opool,
            tc.tile_pool(name="stps", bufs=4, space="PSUM") as stps,
            tc.tile_pool(name="otps", bufs=2, space="PSUM") as otps,
            tc.tile_pool(name="dnps", bufs=2, space="PSUM") as dnps,
        ):
            for m in range(HPC):
                for b in range(B):
                    for c in range(SQC):
                        sq = slice(b * S + c * 512, b * S + (c + 1) * 512)

                        # transposed scores + exp, tile by tile
                        e_t = []
                        for sk in range(SKT):
                            kblk = kTt[m][:, b * S + sk * 128: b * S + (sk + 1) * 128]
                            st_ps = stps.tile([128, 512], F32, tag="st")
                            nc.tensor.matmul(st_ps, kblk, qT[m][:, sq],
                                             start=True, stop=True)
                            e_sb = epool.tile([128, 512], F32R, tag="e")
                            nc.scalar.activation(e_sb, st_ps, AF.Exp, scale=ISCALE)
                            e_t.append(e_sb)

                        # PV + denominator (M=32 ones) accumulation
                        ot_ps = otps.tile([128, 512], F32, tag="ot")
                        dn_ps = dnps.tile([32, 512], F32, tag="dn")
                        for sk in range(SKT):
                            vblk = vN[m][:, b * SKT + sk, :]
                            nc.tensor.matmul(
                                ot_ps, vblk, e_t[sk],
                                start=(sk == 0), stop=(sk == SKT - 1),
                            )
                            nc.tensor.matmul(
                                dn_ps, ones_c.bitcast(F32R), e_t[sk],
                                start=(sk == 0), stop=(sk == SKT - 1),
                            )

                        # denominator -> per-partition reciprocal via PE transpose
                        dn_sb = opool.tile([32, 512], F32, tag="dn_sb")
                        nc.scalar.copy(dn_sb, dn_ps)
                        ot_sb = opool.tile([128, 512], F32, tag="ot_sb")
                        nc.scalar.copy(ot_sb, ot_ps)
                        for blk in range(4):
                            td_ps = stps.tile([128, 32], F32, tag="st", name="td")
                            nc.tensor.transpose(
                                td_ps,
                                dn_sb[0:32, blk * 128:(blk + 1) * 128],
                                ident[0:32, 0:32],
                            )
                            rdt = opool.tile([128, 1], F32, tag="rdt")
                            nc.vector.reciprocal(rdt, td_ps[:, 0:1])
                            tr_ps = stps.tile([128, 128], F32, tag="st", name="tr")
                            nc.tensor.transpose(
                                tr_ps, ot_sb[:, blk * 128:(blk + 1) * 128], ident
                            )
                            o_sb = opool.tile([128, 128], F32, tag="o")
                            nc.vector.tensor_scalar_mul(o_sb, tr_ps, rdt)
                            r0 = b * S + c * 512 + blk * 128
                            nc.sync.dma_start(
                                out_d[r0:r0 + 128, m * 128:(m + 1) * 128], o_sb
                            )

    nc.compile()
    return nc


def _rope_tables():
    inv_freq = 1.0 / (ROPE_BASE ** (np.arange(0, D, 2, dtype=np.float64) / D))
    pos = np.arange(S, dtype=np.float64)
    ang = pos[:, None] * inv_freq[None, :]          # [S, D/2]
    emb = np.concatenate([ang, ang], axis=-1)       # [S, D]
    cosT = np.ascontiguousarray(np.cos(emb).T.astype(np.float32))  # [D, S]
    sinT = np.sin(emb).T.astype(np.float32)
    # swapped + sign-folded so each RoPE half-op reads aligned partitions:
    # t2[0:64] = q[64:128] * sinSw[64:128] (= -sin[0:64])
    # t2[64:128] = q[0:64] * sinSw[0:64]   (= +sin[64:128])
    sinSw = np.concatenate([sinT[64:128], -sinT[0:64]], axis=0)
    return cosT, np.ascontiguousarray(sinSw)


def kernel(hidden_states, Wq, bq, Wk, bk, Wv, bv):
    global LAST_RESULT
    from concourse.bass_utils import run_bass_kernel_spmd

    hs = np.asarray(hidden_states, dtype=np.float32).reshape(BS, H)
    Wq = np.asarray(Wq, dtype=np.float32)
    Wk = np.asarray(Wk, dtype=np.float32)
    Wv = np.asarray(Wv, dtype=np.float32)
    bq = np.asarray(bq, dtype=np.float32)
    bk = np.asarray(bk, dtype=np.float32)
    bv = np.asarray(bv, dtype=np.float32)

    with_bias = bool(np.any(bq) or np.any(bk) or np.any(bv))
    nc = _build_nc(with_bias)

    hsT = np.ascontiguousarray(hs.T)                # [H, BS]
    cosT, sinT = _rope_tables()

    in_maps = []
    for c in range(NCORES):
        ch = slice(c * CH, (c + 1) * CH)
        m = {
            "hsT": hsT,
            "wqT": np.ascontiguousarray(Wq[ch, :].T),
            "wkT": np.ascontiguousarray(Wk[ch, :].T),
            "wvT": np.ascontiguousarray(Wv[ch, :].T),
            "cosT": cosT,
            "sinT": sinT,
        }
        if with_bias:
            m["bq"] = np.ascontiguousarray(bq[None, ch])
            m["bk"] = np.ascontiguousarray(bk[None, ch])
            m["bv"] = np.ascontiguousarray(bv[None, ch])
        else:
            z = np.zeros((1, CH), dtype=np.float32)
            m["bq"] = m["bk"] = m["bv"] = z
        in_maps.append(m)

    res = run_bass_kernel_spmd(nc, in_maps, core_ids=list(range(NCORES)))
    LAST_RESULT = res

    full = np.concatenate([r["out"] for r in res.results], axis=1)  # [BS, H]
    return full.reshape(B, S, H)



# revision 11
# speedup vs baseline: 1.1230x; 1.1230x over previous
"""Multi-head attention (QKV proj + RoPE + softmax attention) on 8 Trainium2
NeuronCores, tensor-parallel over heads (2 heads per core).

v3: bf16 matmul operands, fused projection/attention emission, softmax
denominator via bf16 DVE half-tree + thin [1,512] matmuls (no 32-wide
denominator matmuls), transposed DRAM output ([CH, BS]) with a
broadcast-matmul + GpSimd epilogue (no output/denominator PE transposes).

Contract: kernel(**inputs) takes the FULL unsharded inputs (numpy/jax arrays,
shapes hardcoded below) and returns the FULL [B, S, H] output.
"""

from contextlib import ExitStack

import numpy as np

B, S, H = 2, 2048, 2048
NH, D = 16, 128
ROPE_BASE = 10000.0
NCORES = 8
HPC = NH // NCORES          # heads per core
CH = HPC * D                # output channels per core
BS = B * S                  # flattened tokens
KT = H // 128               # contraction k-tiles
NCH = BS // 512             # 512-wide token chunks
SKT = S // 128              # score k-tiles per sequence
SQC = S // 512              # query chunks per sequence
EBIAS = 2.0                 # exp(score - EBIAS): cancels in softmax division

LAST_RESULT = None          # BassKernelResults of the most recent run (for test.py)


def _build_nc(with_bias):
    import concourse.mybir as mybir
    import concourse.tile as tile
    from concourse import bacc
    from concourse.masks import make_identity

    F32 = mybir.dt.float32
    F32R = mybir.dt.float32r
    BF = mybir.dt.bfloat16
    AF = mybir.ActivationFunctionType
    ALU = mybir.AluOpType
    ISCALE = float(1.0 / np.sqrt(D))

    nc = bacc.Bacc("TRN2", debug=False, enable_partition_id=False)

    hsT_d = nc.dram_tensor("hsT", [H, BS], BF, kind="ExternalInput").ap()
    wT_d = {
        p: nc.dram_tensor(f"w{p}T", [H, CH], BF, kind="ExternalInput").ap()
        for p in "qkv"
    }
    b_d = {
        p: nc.dram_tensor(f"b{p}", [1, CH], BF, kind="ExternalInput").ap()
        for p in "qkv"
    }
    cos_d = nc.dram_tensor("cosT", [D, S], F32, kind="ExternalInput").ap()
    sin_d = nc.dram_tensor("sinT", [D, S], F32, kind="ExternalInput").ap()
    out_d = nc.dram_tensor("out", [CH, BS], F32, kind="ExternalOutput").ap()

    with tile.TileContext(nc) as tc, ExitStack() as ctx:
        # ---- persistent state ----
        persist = ctx.enter_context(tc.tile_pool(name="persist", bufs=1))
        qT = [persist.tile([128, BS], BF, tag=f"qT{m}", name=f"qT{m}") for m in range(HPC)]
        kTt = [persist.tile([128, BS], BF, tag=f"kT{m}", name=f"kT{m}") for m in range(HPC)]
        vN = [persist.tile([128, BS // 128, D], BF, tag=f"v{m}", name=f"vn{m}") for m in range(HPC)]

        consts = ctx.enter_context(tc.tile_pool(name="consts", bufs=1))
        identB = consts.tile([128, 128], BF, tag="identB")
        make_identity(nc, identB)
        onesK = consts.tile([128, 1], BF, tag="onesK")
        nc.vector.memset(onesK, 1.0)
        ebias_sb = consts.tile([128, 1], F32, tag="ebias")
        nc.vector.memset(ebias_sb, -EBIAS)
        if with_bias:
            ones_row = consts.tile([1, 512], BF, tag="ones_row")
            nc.vector.memset(ones_row, 1.0)
            b_sb = {}
            for p in "qkv":
                b_sb[p] = consts.tile([1, CH], BF, tag=f"b{p}", name=f"b{p}sb")
                nc.sync.dma_start(b_sb[p], b_d[p])

        # ---- pools (all live for the whole fused kernel) ----
        wpool = ctx.enter_context(tc.tile_pool(name="wpool", bufs=1))
        tabs = ctx.enter_context(tc.tile_pool(name="tabs", bufs=1))
        hstp = ctx.enter_context(tc.tile_pool(name="hstp", bufs=2))
        ropet = ctx.enter_context(tc.tile_pool(name="ropet", bufs=3))
        vtp = ctx.enter_context(tc.tile_pool(name="vtp", bufs=2))
        epool = ctx.enter_context(tc.tile_pool(name="epool", bufs=2))
        esump = ctx.enter_context(tc.tile_pool(name="esump", bufs=4))
        recp = ctx.enter_context(tc.tile_pool(name="recp", bufs=2))
        opool = ctx.enter_context(tc.tile_pool(name="opool", bufs=2))

        prjp = ctx.enter_context(tc.tile_pool(name="prjp", bufs=2, space="PSUM"))
        stp = ctx.enter_context(tc.tile_pool(name="stp", bufs=2, space="PSUM"))
        otp = ctx.enter_context(tc.tile_pool(name="otp", bufs=1, space="PSUM"))
        dnp = ctx.enter_context(tc.tile_pool(name="dnp", bufs=1, space="PSUM"))

        # ---- table + weight loads ----
        cos_sb = tabs.tile([D, S], F32, tag="cos")
        sin_sb = tabs.tile([D, S], F32, tag="sin")
        nc.sync.dma_start(cos_sb, cos_d)
        nc.sync.dma_start(sin_sb, sin_d)
        w_sb = {}
        for p in "qkv":
            w_sb[p] = wpool.tile([128, KT, CH], BF, tag=f"w{p}", name=f"w{p}sb")
        w_r = {p: wT_d[p].rearrange("(k p) c -> p k c", p=128) for p in "qkv"}
        for k in range(KT):
            for p in "qkv":
                nc.sync.dma_start(w_sb[p][:, k, :], w_r[p][:, k, :])

        hsT_r = hsT_d.rearrange("(k p) t -> p k t", p=128)
        hs_tiles = {}

        def dma_hs(n):
            t = hstp.tile([128, KT, 512], BF, tag="hs", name=f"hs{n}")
            nc.sync.dma_start(t, hsT_r[:, :, n * 512:(n + 1) * 512])
            hs_tiles[n] = t

        # deferred PE work (v-transposes, attention epilogues) flushed after
        # the next bulk of independent PE instructions has been emitted
        deferred = []

        def flush():
            for fn in deferred:
                fn()
            deferred.clear()

        # ---------------- projection chunk ----------------
        def emit_chunk(n):
            tok = slice(n * 512, (n + 1) * 512)
            pos = slice((n % SQC) * 512, (n % SQC + 1) * 512)
            hs_t = hs_tiles[n]
            for ci, (p, mi) in enumerate(
                [("q", 0), ("k", 0), ("v", 0), ("q", 1), ("k", 1), ("v", 1)]
            ):
                ps = prjp.tile([128, 512], F32, tag="prj", name=f"pj{n}{ci}")
                for k in range(KT):
                    nc.tensor.matmul(
                        ps,
                        w_sb[p][:, k, mi * 128:(mi + 1) * 128],
                        hs_t[:, k, :],
                        start=(k == 0),
                        stop=(k == KT - 1) and not with_bias,
                    )
                if with_bias:
                    nc.tensor.matmul(
                        ps,
                        b_sb[p][:, mi * 128:(mi + 1) * 128],
                        ones_row,
                        start=False,
                        stop=True,
                    )
                flush()
                if p == "v":
                    # psum -> bf16 sbuf, then (deferred) PE-transpose into
                    # natural [token, d] layout
                    vt = vtp.tile([128, 512], BF, tag="vt")
                    nc.vector.tensor_copy(vt, ps)

                    def vtrans(vt=vt, mi=mi, n=n):
                        tp = stp.tile([128, 2, 512], F32, tag="st", name=f"vt{n}{mi}")
                        for blk in range(4):
                            dst8 = tp[:, 0, blk * 64:(blk + 1) * 64].bitcast(BF)
                            nc.tensor.transpose(
                                dst8, vt[:, blk * 128:(blk + 1) * 128], identB
                            )
                            nc.vector.tensor_copy(vN[mi][:, n * 4 + blk, :], dst8)

                    deferred.append(vtrans)
                else:
                    dst = qT if p == "q" else kTt
                    # RoPE: t1 = ps*cos (gpsimd), t2 = rot(ps)*sin (DVE),
                    # out = t1 + t2 -> bf16 persist (DVE)
                    t1 = ropet.tile([128, 512], F32, tag="t1")
                    nc.vector.tensor_tensor(t1, ps, cos_sb[:, pos], op=ALU.mult)
                    t2 = ropet.tile([128, 512], F32, tag="t2")
                    nc.vector.tensor_tensor(
                        t2[0:64], ps[64:128], sin_sb[64:128, pos], op=ALU.mult
                    )
                    nc.vector.tensor_tensor(
                        t2[64:128], ps[0:64], sin_sb[0:64, pos], op=ALU.mult
                    )
                    nc.vector.tensor_tensor(dst[mi][:, tok], t1, t2, op=ALU.add)

        # ---------------- attention iteration ----------------
        def emit_att(m, b, c):
            sq = slice(b * S + c * 512, b * S + (c + 1) * 512)
            e_t = epool.tile([128, SKT, 512], BF, tag="e", name=f"e{m}{b}{c}")
            ot = otp.tile([128, 512], F32, tag="ot", name=f"ot{m}{b}{c}")
            dn = dnp.tile([1, 512], F32, tag="dn", name=f"dn{m}{b}{c}")
            esA = esump.tile([128, 4, 512], BF, tag="es", name=f"esA{m}{b}{c}")
            esB = esump.tile([128, 4, 512], BF, tag="es", name=f"esB{m}{b}{c}")

            for t in range(8):
                st = stp.tile([128, 2, 512], F32, tag="st", name=f"st{m}{b}{c}{t}")
                for half in range(2):
                    sk = 2 * t + half
                    kblk = kTt[m][:, b * S + sk * 128: b * S + (sk + 1) * 128]
                    nc.tensor.matmul(st[:, half, :], kblk, qT[m][:, sq],
                                     start=True, stop=True)
                if t == 1:
                    flush()
                nc.scalar.activation(
                    e_t[:, 2 * t:2 * t + 2, :], st, AF.Exp,
                    bias=ebias_sb, scale=ISCALE,
                )
                # PV matmuls for the previous tile's pair (let exp run ahead)
                if t >= 1:
                    for sk in (2 * t - 2, 2 * t - 1):
                        nc.tensor.matmul(
                            ot, vN[m][:, b * SKT + sk, :], e_t[:, sk, :],
                            start=(sk == 0), stop=False,
                        )
                if t == 3:
                    nc.vector.tensor_tensor(
                        esA, e_t[:, 0:4, :], e_t[:, 4:8, :], op=ALU.add
                    )
            for sk in (SKT - 2, SKT - 1):
                nc.tensor.matmul(
                    ot, vN[m][:, b * SKT + sk, :], e_t[:, sk, :],
                    start=False, stop=(sk == SKT - 1),
                )
            nc.vector.tensor_tensor(
                esB, e_t[:, 8:12, :], e_t[:, 12:16, :], op=ALU.add
            )
            for j in range(4):
                nc.tensor.matmul(dn, onesK, esA[:, j, :],
                                 start=(j == 0), stop=False)
            for j in range(4):
                nc.tensor.matmul(dn, onesK, esB[:, j, :],
                                 start=False, stop=(j == 3))
            rec_sb = recp.tile([1, 512], F32, tag="rec", name=f"rc{m}{b}{c}")
            nc.vector.reciprocal(rec_sb, dn)
            rec_bc = recp.tile([128, 512], F32, tag="recb", name=f"rb{m}{b}{c}")
            nc.gpsimd.partition_broadcast(rec_bc, rec_sb)

            def epilogue(m=m, b=b, c=c, ot=ot, rec_bc=rec_bc):
                o_sb = opool.tile([128, 512], F32, tag="o", name=f"o{m}{b}{c}")
                nc.vector.tensor_tensor(o_sb, ot, rec_bc, op=ALU.mult)
                nc.sync.dma_start(
                    out_d[m * 128:(m + 1) * 128, b * S + c * 512: b * S + (c + 1) * 512],
                    o_sb,
                )

            deferred.append(epilogue)

        # ---------------- fused schedule ----------------
        att_iters = {
            b: [(m, c) for c in range(SQC) for m in range(HPC)] for b in range(B)
        }
        dma_hs(0)
        for n in range(NCH):
            if n + 1 < NCH:
                dma_hs(n + 1)
            emit_chunk(n)
            if n >= SQC:
                # interleave two batch-0 attention iterations per late chunk
                for m, c in att_iters[0][2 * (n - SQC): 2 * (n - SQC) + 2]:
                    emit_att(m, 0, c)
        for m, c in att_iters[0][2 * (NCH - 1 - SQC) + 2:]:
            emit_att(m, 0, c)
        for m, c in att_iters[1]:
            emit_att(m, 1, c)
        flush()

    nc.compile()
    return nc


def _rope_tables():
    inv_freq = 1.0 / (ROPE_BASE ** (np.arange(0, D, 2, dtype=np.float64) / D))
    pos = np.arange(S, dtype=np.float64)
    ang = pos[:, None] * inv_freq[None, :]          # [S, D/2]
    emb = np.concatenate([ang, ang], axis=-1)       # [S, D]
    cosT = np.ascontiguousarray(np.cos(emb).T.astype(np.float32))  # [D, S]
    sinT = np.sin(emb).T.astype(np.float32)
    # swapped + sign-folded so each RoPE half-op reads aligned partitions:
    # t2[0:64] = q[64:128] * sinSw[64:128] (= -sin[0:64])
    # t2[64:128] = q[0:64] * sinSw[0:64]   (= +sin[64:128])
    sinSw = np.concatenate([sinT[64:128], -sinT[0:64]], axis=0)
    return cosT, np.ascontiguousarray(sinSw)


def kernel(hidden_states, Wq, bq, Wk, bk, Wv, bv):
    global LAST_RESULT
    import ml_dtypes
    from concourse.bass_utils import run_bass_kernel_spmd

    BF = ml_dtypes.bfloat16

    hs = np.asarray(hidden_states, dtype=np.float32).reshape(BS, H)
    Wq = np.asarray(Wq, dtype=np.float32)
    Wk = np.asarray(Wk, dtype=np.float32)
    Wv = np.asarray(Wv, dtype=np.float32)
    bq = np.asarray(bq, dtype=np.float32)
    bk = np.asarray(bk, dtype=np.float32)
    bv = np.asarray(bv, dtype=np.float32)

    with_bias = bool(np.any(bq) or np.any(bk) or np.any(bv))
    nc = _build_nc(with_bias)

    hsT = np.ascontiguousarray(hs.T).astype(BF)     # [H, BS]
    cosT, sinT = _rope_tables()

    in_maps = []
    for c in range(NCORES):
        ch = slice(c * CH, (c + 1) * CH)
        m = {
            "hsT": hsT,
            "wqT": np.ascontiguousarray(Wq[ch, :].T).astype(BF),
            "wkT": np.ascontiguousarray(Wk[ch, :].T).astype(BF),
            "wvT": np.ascontiguousarray(Wv[ch, :].T).astype(BF),
            "cosT": cosT,
            "sinT": sinT,
        }
        if with_bias:
            m["bq"] = np.ascontiguousarray(bq[None, ch]).astype(BF)
            m["bk"] = np.ascontiguousarray(bk[None, ch]).astype(BF)
            m["bv"] = np.ascontiguousarray(bv[None, ch]).astype(BF)
        else:
            z = np.zeros((1, CH), dtype=BF)
            m["bq"] = m["bk"] = m["bv"] = z
        in_maps.append(m)

    res = run_bass_kernel_spmd(nc, in_maps, core_ids=list(range(NCORES)))
    LAST_RESULT = res

    full = np.concatenate([r["out"] for r in res.results], axis=0)  # [H, BS]
    return np.ascontiguousarray(full.T).reshape(B, S, H)


# revision 20
# speedup vs baseline: 1.2060x; 1.0739x over previous
"""Multi-head attention (QKV proj + RoPE + softmax attention) on 8 Trainium2
NeuronCores, tensor-parallel over heads (2 heads per core).

v3: bf16 matmul operands, fused projection/attention emission, softmax
denominator via bf16 DVE half-tree + thin [1,512] matmuls (no 32-wide
denominator matmuls), transposed DRAM output ([CH, BS]) with a
broadcast-matmul + GpSimd epilogue (no output/denominator PE transposes).

Contract: kernel(**inputs) takes the FULL unsharded inputs (numpy/jax arrays,
shapes hardcoded below) and returns the FULL [B, S, H] output.
"""

from contextlib import ExitStack

import numpy as np

B, S, H = 2, 2048, 2048
NH, D = 16, 128
ROPE_BASE = 10000.0
NCORES = 8
HPC = NH // NCORES          # heads per core
CH = HPC * D                # output channels per core
BS = B * S                  # flattened tokens
KT = H // 128               # contraction k-tiles
NCH = BS // 512             # 512-wide token chunks
SKT = S // 128              # score k-tiles per sequence
SQC = S // 512              # query chunks per sequence
EBIAS = 2.0                 # exp(score - EBIAS): cancels in softmax division

LAST_RESULT = None          # BassKernelResults of the most recent run (for test.py)


def _build_nc(with_bias):
    import concourse.mybir as mybir
    import concourse.tile as tile
    from concourse import bacc
    from concourse.masks import make_identity

    F32 = mybir.dt.float32
    F32R = mybir.dt.float32r
    BF = mybir.dt.bfloat16
    AF = mybir.ActivationFunctionType
    ALU = mybir.AluOpType
    ISCALE = float(1.0 / np.sqrt(D))

    nc = bacc.Bacc("TRN2", debug=False, enable_partition_id=False)

    hsT_d = nc.dram_tensor("hsT", [H, BS], BF, kind="ExternalInput").ap()
    wT_d = {
        p: nc.dram_tensor(f"w{p}T", [H, CH], BF, kind="ExternalInput").ap()
        for p in "qkv"
    }
    b_d = {
        p: nc.dram_tensor(f"b{p}", [1, CH], BF, kind="ExternalInput").ap()
        for p in "qkv"
    }
    cos_d = nc.dram_tensor("cosT", [D, S], F32, kind="ExternalInput").ap()
    sin_d = nc.dram_tensor("sinT", [D, S], F32, kind="ExternalInput").ap()
    out_d = nc.dram_tensor("out", [CH, BS], F32, kind="ExternalOutput").ap()

    with tile.TileContext(nc) as tc, ExitStack() as ctx:
        # ---- persistent state ----
        persist = ctx.enter_context(tc.tile_pool(name="persist", bufs=1))
        qT = [persist.tile([128, BS], BF, tag=f"qT{m}", name=f"qT{m}") for m in range(HPC)]
        kTt = [persist.tile([128, BS], BF, tag=f"kT{m}", name=f"kT{m}") for m in range(HPC)]
        vN = [persist.tile([128, BS // 128, D], BF, tag=f"v{m}", name=f"vn{m}") for m in range(HPC)]

        consts = ctx.enter_context(tc.tile_pool(name="consts", bufs=1))
        identB = consts.tile([128, 128], BF, tag="identB")
        make_identity(nc, identB)
        identF = consts.tile([128, 128], F32, tag="identF")
        make_identity(nc, identF)
        onesK = consts.tile([128, 1], BF, tag="onesK")
        nc.vector.memset(onesK, 1.0)
        ebias_sb = consts.tile([128, 1], F32, tag="ebias")
        nc.vector.memset(ebias_sb, -EBIAS)
        if with_bias:
            ones_row = consts.tile([1, 512], BF, tag="ones_row")
            nc.vector.memset(ones_row, 1.0)
            b_sb = {}
            for p in "qkv":
                b_sb[p] = consts.tile([1, CH], BF, tag=f"b{p}", name=f"b{p}sb")
                nc.sync.dma_start(b_sb[p], b_d[p])

        # ---- pools (all live for the whole fused kernel) ----
        wpool = ctx.enter_context(tc.tile_pool(name="wpool", bufs=1))
        tabs = ctx.enter_context(tc.tile_pool(name="tabs", bufs=1))
        hstp = ctx.enter_context(tc.tile_pool(name="hstp", bufs=2))
        ropet = ctx.enter_context(tc.tile_pool(name="ropet", bufs=2))
        vtp = ctx.enter_context(tc.tile_pool(name="vtp", bufs=2))
        epool = ctx.enter_context(tc.tile_pool(name="epool", bufs=2))
        esump = ctx.enter_context(tc.tile_pool(name="esump", bufs=4))
        es2p = ctx.enter_context(tc.tile_pool(name="es2p", bufs=2))
        recp = ctx.enter_context(tc.tile_pool(name="recp", bufs=2))
        opool = ctx.enter_context(tc.tile_pool(name="opool", bufs=2))

        prjp = ctx.enter_context(tc.tile_pool(name="prjp", bufs=2, space="PSUM"))
        stp = ctx.enter_context(tc.tile_pool(name="stp", bufs=2, space="PSUM"))
        otp = ctx.enter_context(tc.tile_pool(name="otp", bufs=2, space="PSUM"))

        # ---- table + weight loads ----
        cos_sb = tabs.tile([D, S], F32, tag="cos")
        sin_sb = tabs.tile([D, S], F32, tag="sin")
        for c in range(SQC):
            cs = slice(c * 512, (c + 1) * 512)
            nc.sync.dma_start(cos_sb[:, cs], cos_d[:, cs])
            nc.sync.dma_start(sin_sb[:, cs], sin_d[:, cs])
        w_sb = {}
        for p in "qkv":
            w_sb[p] = wpool.tile([128, KT, CH], BF, tag=f"w{p}", name=f"w{p}sb")
        w_r = {p: wT_d[p].rearrange("(k p) c -> p k c", p=128) for p in "qkv"}
        for k in range(KT):
            for p in "qkv":
                nc.sync.dma_start(w_sb[p][:, k, :], w_r[p][:, k, :])

        hsT_r = hsT_d.rearrange("(k p) t -> p k t", p=128)
        hs_tiles = {}

        def dma_hs(n):
            t = hstp.tile([128, KT, 512], BF, tag="hs", name=f"hs{n}")
            # per-k transfers spread across DMA queues (a single dma_start
            # serializes ~2MB onto one ~20GB/s queue)
            for k in range(KT):
                nc.sync.dma_start(t[:, k, :], hsT_r[:, k, n * 512:(n + 1) * 512])
            hs_tiles[n] = t

        # deferred PE work (v-transposes, attention epilogues) flushed after
        # the next bulk of independent PE instructions has been emitted
        deferred = []

        def flush():
            for fn in deferred:
                fn()
            deferred.clear()

        # ---------------- projection chunk ----------------
        def emit_chunk(n):
            tok = slice(n * 512, (n + 1) * 512)
            pos = slice((n % SQC) * 512, (n % SQC + 1) * 512)
            hs_t = hs_tiles[n]
            for ci, (p, mi) in enumerate(
                [("q", 0), ("k", 0), ("v", 0), ("q", 1), ("k", 1), ("v", 1)]
            ):
                ps = prjp.tile([128, 512], F32, tag="prj", name=f"pj{n}{ci}")
                for k in range(KT):
                    nc.tensor.matmul(
                        ps,
                        w_sb[p][:, k, mi * 128:(mi + 1) * 128],
                        hs_t[:, k, :],
                        start=(k == 0),
                        stop=(k == KT - 1) and not with_bias,
                    )
                if with_bias:
                    nc.tensor.matmul(
                        ps,
                        b_sb[p][:, mi * 128:(mi + 1) * 128],
                        ones_row,
                        start=False,
                        stop=True,
                    )
                flush()
                if p == "v":
                    # psum -> bf16 sbuf, then (deferred) PE-transpose into
                    # natural [token, d] layout
                    vt = vtp.tile([128, 512], BF, tag="vt")
                    nc.vector.tensor_copy(vt, ps)

                    def vtrans(vt=vt, mi=mi, n=n):
                        tp = stp.tile([128, 2, 512], F32, tag="st", name=f"vt{n}{mi}")
                        for blk in range(4):
                            dst8 = tp[:, 0, blk * 64:(blk + 1) * 64].bitcast(BF)
                            nc.tensor.transpose(
                                dst8, vt[:, blk * 128:(blk + 1) * 128], identB
                            )
                            nc.vector.tensor_copy(vN[mi][:, n * 4 + blk, :], dst8)

                    deferred.append(vtrans)
                else:
                    dst = qT if p == "q" else kTt
                    # RoPE: t1 = ps*cos (gpsimd), t2 = rot(ps)*sin (DVE),
                    # out = t1 + t2 -> bf16 persist (DVE)
                    t1 = ropet.tile([128, 512], F32, tag="t1")
                    nc.vector.tensor_tensor(t1, ps, cos_sb[:, pos], op=ALU.mult)
                    t2 = ropet.tile([128, 512], F32, tag="t2")
                    nc.vector.tensor_tensor(
                        t2[0:64], ps[64:128], sin_sb[64:128, pos], op=ALU.mult
                    )
                    nc.vector.tensor_tensor(
                        t2[64:128], ps[0:64], sin_sb[0:64, pos], op=ALU.mult
                    )
                    nc.vector.tensor_tensor(dst[mi][:, tok], t1, t2, op=ALU.add)

        # ---------------- attention iteration ----------------
        def emit_att(m, b, c):
            sq = slice(b * S + c * 512, b * S + (c + 1) * 512)
            e_t = epool.tile([128, SKT, 512], BF, tag="e", name=f"e{m}{b}{c}")
            ot = otp.tile([128, 512], F32, tag="ot", name=f"ot{m}{b}{c}")
            esA = esump.tile([128, 4, 512], BF, tag="es", name=f"esA{m}{b}{c}")
            esB = esump.tile([128, 4, 512], BF, tag="es", name=f"esB{m}{b}{c}")

            for t in range(8):
                st = stp.tile([128, 2, 512], F32, tag="st", name=f"st{m}{b}{c}{t}")
                for half in range(2):
                    sk = 2 * t + half
                    kblk = kTt[m][:, b * S + sk * 128: b * S + (sk + 1) * 128]
                    nc.tensor.matmul(st[:, half, :], kblk, qT[m][:, sq],
                                     start=True, stop=True)
                if t == 1:
                    flush()
                nc.scalar.activation(
                    e_t[:, 2 * t:2 * t + 2, :], st, AF.Exp,
                    bias=ebias_sb, scale=ISCALE,
                )
                # PV matmuls for the previous tile's pair (let exp run ahead)
                if t >= 1:
                    for sk in (2 * t - 2, 2 * t - 1):
                        nc.tensor.matmul(
                            ot, vN[m][:, b * SKT + sk, :], e_t[:, sk, :],
                            start=(sk == 0), stop=False,
                        )
                if t == 3:
                    nc.vector.tensor_tensor(
                        esA, e_t[:, 0:4, :], e_t[:, 4:8, :], op=ALU.add
                    )
            for sk in (SKT - 2, SKT - 1):
                nc.tensor.matmul(
                    ot, vN[m][:, b * SKT + sk, :], e_t[:, sk, :],
                    start=False, stop=(sk == SKT - 1),
                )
            nc.vector.tensor_tensor(
                esB, e_t[:, 8:12, :], e_t[:, 12:16, :], op=ALU.add
            )
            # finish the tree: e_sum[key, q] = sum over the 16 key-tiles
            t2 = es2p.tile([128, 4, 512], BF, tag="es2", name=f"t2{m}{b}{c}")
            nc.vector.tensor_tensor(t2, esA, esB, op=ALU.add)
            t3 = es2p.tile([128, 2, 512], BF, tag="es3", name=f"t3{m}{b}{c}")
            nc.vector.tensor_tensor(t3, t2[:, 0:2, :], t2[:, 2:4, :], op=ALU.add)
            esum = es2p.tile([128, 512], BF, tag="esum", name=f"es{m}{b}{c}")
            nc.vector.tensor_tensor(esum, t3[:, 0, :], t3[:, 1, :], op=ALU.add)
            # denominators with queries on partitions: dn_t[q, qb] so the
            # reciprocal runs 128-wide (a [1,512] DVE reciprocal costs 3.3us)
            dn_tile = stp.tile([128, 2, 512], F32, tag="st", name=f"dn{m}{b}{c}")
            dn_t = dn_tile[:, 0, 0:4]
            for qb in range(4):
                nc.tensor.matmul(
                    dn_t[:, qb:qb + 1], esum[:, qb * 128:(qb + 1) * 128], onesK,
                    start=True, stop=True,
                )
            rec_sb = recp.tile([128, 4], F32, tag="rec", name=f"rc{m}{b}{c}")
            nc.vector.reciprocal(rec_sb, dn_t)

            def epilogue(m=m, b=b, c=c, ot=ot, rec_sb=rec_sb):
                rp = stp.tile([128, 2, 512], F32, tag="st", name=f"rp{m}{b}{c}")
                for qb in range(4):
                    nc.tensor.transpose(
                        rp[0:1, 0, qb * 128:(qb + 1) * 128],
                        rec_sb[:, qb:qb + 1], identF,
                    )
                rec_row = recp.tile([1, 512], F32, tag="recr", name=f"rr{m}{b}{c}")
                nc.vector.tensor_copy(rec_row, rp[0:1, 0, :])
                rec_bc = recp.tile([128, 512], F32, tag="recb", name=f"rb{m}{b}{c}")
                nc.gpsimd.partition_broadcast(rec_bc, rec_row)
                o_sb = opool.tile([128, 512], F32, tag="o", name=f"o{m}{b}{c}")
                nc.vector.tensor_tensor(o_sb, ot, rec_bc, op=ALU.mult)
                nc.sync.dma_start(
                    out_d[m * 128:(m + 1) * 128, b * S + c * 512: b * S + (c + 1) * 512],
                    o_sb,
                )

            deferred.append(epilogue)

        # ---------------- fused schedule ----------------
        att_iters = {
            b: [(m, c) for c in range(SQC) for m in range(HPC)] for b in range(B)
        }
        dma_hs(0)
        for n in range(NCH):
            if n + 1 < NCH:
                dma_hs(n + 1)
            emit_chunk(n)
            if n >= SQC:
                # interleave two batch-0 attention iterations per late chunk
                for m, c in att_iters[0][2 * (n - SQC): 2 * (n - SQC) + 2]:
                    emit_att(m, 0, c)
        for m, c in att_iters[0][2 * (NCH - 1 - SQC) + 2:]:
            emit_att(m, 0, c)
        for m, c in att_iters[1]:
            emit_att(m, 1, c)
        flush()

    nc.compile()
    return nc


def _rope_tables():
    inv_freq = 1.0 / (ROPE_BASE ** (np.arange(0, D, 2, dtype=np.float64) / D))
    pos = np.arange(S, dtype=np.float64)
    ang = pos[:, None] * inv_freq[None, :]          # [S, D/2]
    emb = np.concatenate([ang, ang], axis=-1)       # [S, D]
    cosT = np.ascontiguousarray(np.cos(emb).T.astype(np.float32))  # [D, S]
    sinT = np.sin(emb).T.astype(np.float32)
    # swapped + sign-folded so each RoPE half-op reads aligned partitions:
    # t2[0:64] = q[64:128] * sinSw[64:128] (= -sin[0:64])
    # t2[64:128] = q[0:64] * sinSw[0:64]   (= +sin[64:128])
    sinSw = np.concatenate([sinT[64:128], -sinT[0:64]], axis=0)
    return cosT, np.ascontiguousarray(sinSw)


def kernel(hidden_states, Wq, bq, Wk, bk, Wv, bv):
    global LAST_RESULT
    import ml_dtypes
    from concourse.bass_utils import run_bass_kernel_spmd

    BF = ml_dtypes.bfloat16

    hs = np.asarray(hidden_states, dtype=np.float32).reshape(BS, H)
    Wq = np.asarray(Wq, dtype=np.float32)
    Wk = np.asarray(Wk, dtype=np.float32)
    Wv = np.asarray(Wv, dtype=np.float32)
    bq = np.asarray(bq, dtype=np.float32)
    bk = np.asarray(bk, dtype=np.float32)
    bv = np.asarray(bv, dtype=np.float32)

    with_bias = bool(np.any(bq) or np.any(bk) or np.any(bv))
    nc = _build_nc(with_bias)

    hsT = np.ascontiguousarray(hs.T).astype(BF)     # [H, BS]
    cosT, sinT = _rope_tables()

    in_maps = []
    for c in range(NCORES):
        ch = slice(c * CH, (c + 1) * CH)
        m = {
            "hsT": hsT,
            "wqT": np.ascontiguousarray(Wq[ch, :].T).astype(BF),
            "wkT": np.ascontiguousarray(Wk[ch, :].T).astype(BF),
            "wvT": np.ascontiguousarray(Wv[ch, :].T).astype(BF),
            "cosT": cosT,
            "sinT": sinT,
        }
        if with_bias:
            m["bq"] = np.ascontiguousarray(bq[None, ch]).astype(BF)
            m["bk"] = np.ascontiguousarray(bk[None, ch]).astype(BF)
            m["bv"] = np.ascontiguousarray(bv[None, ch]).astype(BF)
        else:
            z = np.zeros((1, CH), dtype=BF)
            m["bq"] = m["bk"] = m["bv"] = z
        in_maps.append(m)

    res = run_bass_kernel_spmd(nc, in_maps, core_ids=list(range(NCORES)))
    LAST_RESULT = res

    full = np.concatenate([r["out"] for r in res.results], axis=0)  # [H, BS]
    return np.ascontiguousarray(full.T).reshape(B, S, H)


# revision 24
# speedup vs baseline: 1.3061x; 1.0830x over previous
"""Multi-head attention (QKV proj + RoPE + softmax attention) on 8 Trainium2
NeuronCores, tensor-parallel over heads (2 heads per core).

v3: bf16 matmul operands, fused projection/attention emission, softmax
denominator via bf16 DVE half-tree + thin [1,512] matmuls (no 32-wide
denominator matmuls), transposed DRAM output ([CH, BS]) with a
broadcast-matmul + GpSimd epilogue (no output/denominator PE transposes).

Contract: kernel(**inputs) takes the FULL unsharded inputs (numpy/jax arrays,
shapes hardcoded below) and returns the FULL [B, S, H] output.
"""

from contextlib import ExitStack

import numpy as np

B, S, H = 2, 2048, 2048
NH, D = 16, 128
ROPE_BASE = 10000.0
NCORES = 8
HPC = NH // NCORES          # heads per core
CH = HPC * D                # output channels per core
BS = B * S                  # flattened tokens
KT = H // 128               # contraction k-tiles
NCH = BS // 512             # 512-wide token chunks
SKT = S // 128              # score k-tiles per sequence
SQC = S // 512              # query chunks per sequence
EBIAS = 2.0                 # exp(score - EBIAS): cancels in softmax division

LAST_RESULT = None          # BassKernelResults of the most recent run (for test.py)


def _build_nc(with_bias):
    import concourse.mybir as mybir
    import concourse.tile as tile
    from concourse import bacc
    from concourse.masks import make_identity

    F32 = mybir.dt.float32
    F32R = mybir.dt.float32r
    BF = mybir.dt.bfloat16
    AF = mybir.ActivationFunctionType
    ALU = mybir.AluOpType
    ISCALE = float(1.0 / np.sqrt(D))

    nc = bacc.Bacc("TRN2", debug=False, enable_partition_id=False)

    hsT_d = nc.dram_tensor("hsT", [H, BS], BF, kind="ExternalInput").ap()
    wT_d = {
        p: nc.dram_tensor(f"w{p}T", [H, CH], BF, kind="ExternalInput").ap()
        for p in "qkv"
    }
    b_d = {
        p: nc.dram_tensor(f"b{p}", [1, CH], BF, kind="ExternalInput").ap()
        for p in "qkv"
    }
    cos_d = nc.dram_tensor("cosT", [D, S], F32, kind="ExternalInput").ap()
    sin_d = nc.dram_tensor("sinT", [D, S], F32, kind="ExternalInput").ap()
    out_d = nc.dram_tensor("out", [CH, BS], F32, kind="ExternalOutput").ap()

    with tile.TileContext(nc) as tc, ExitStack() as ctx:
        # ---- persistent state ----
        persist = ctx.enter_context(tc.tile_pool(name="persist", bufs=1))
        qT = [persist.tile([128, BS], BF, tag=f"qT{m}", name=f"qT{m}") for m in range(HPC)]
        kTt = [persist.tile([128, BS], BF, tag=f"kT{m}", name=f"kT{m}") for m in range(HPC)]
        vN = [persist.tile([128, BS // 128, D], BF, tag=f"v{m}", name=f"vn{m}") for m in range(HPC)]

        consts = ctx.enter_context(tc.tile_pool(name="consts", bufs=1))
        identB = consts.tile([128, 128], BF, tag="identB")
        make_identity(nc, identB)
        identF = consts.tile([128, 128], F32, tag="identF")
        make_identity(nc, identF)
        onesK = consts.tile([128, 1], BF, tag="onesK")
        nc.vector.memset(onesK, 1.0)
        ebias_sb = consts.tile([128, 1], F32, tag="ebias")
        nc.vector.memset(ebias_sb, -EBIAS)
        if with_bias:
            ones_row = consts.tile([1, 512], BF, tag="ones_row")
            nc.vector.memset(ones_row, 1.0)
            b_sb = {}
            for p in "qkv":
                b_sb[p] = consts.tile([1, CH], BF, tag=f"b{p}", name=f"b{p}sb")
                nc.sync.dma_start(b_sb[p], b_d[p])

        # ---- pools (all live for the whole fused kernel) ----
        wpool = ctx.enter_context(tc.tile_pool(name="wpool", bufs=1))
        tabs = ctx.enter_context(tc.tile_pool(name="tabs", bufs=1))
        hstp = ctx.enter_context(tc.tile_pool(name="hstp", bufs=2))
        ropet = ctx.enter_context(tc.tile_pool(name="ropet", bufs=2))
        vtp = ctx.enter_context(tc.tile_pool(name="vtp", bufs=2))
        epool = ctx.enter_context(tc.tile_pool(name="epool", bufs=2))
        esump = ctx.enter_context(tc.tile_pool(name="esump", bufs=4))
        es2p = ctx.enter_context(tc.tile_pool(name="es2p", bufs=2))
        recp = ctx.enter_context(tc.tile_pool(name="recp", bufs=2))
        opool = ctx.enter_context(tc.tile_pool(name="opool", bufs=2))

        prjp = ctx.enter_context(tc.tile_pool(name="prjp", bufs=2, space="PSUM"))
        stp = ctx.enter_context(tc.tile_pool(name="stp", bufs=2, space="PSUM"))
        otp = ctx.enter_context(tc.tile_pool(name="otp", bufs=2, space="PSUM"))

        # ---- table + weight loads ----
        cos_sb = tabs.tile([D, S], F32, tag="cos")
        sin_sb = tabs.tile([D, S], F32, tag="sin")
        w_sb = {}
        for p in "qkv":
            w_sb[p] = wpool.tile([128, KT, CH], BF, tag=f"w{p}", name=f"w{p}sb")
        w_r = {p: wT_d[p].rearrange("(k p) c -> p k c", p=128) for p in "qkv"}

        hsT_r = hsT_d.rearrange("(k p) t -> p k t", p=128)
        hs_tiles = {}

        def dma_hs(n, wload=False):
            # per-k transfers spread across DMA queues (a single dma_start
            # serializes ~2MB onto one ~20GB/s queue); DMAs are emitted in
            # first-use order so chunk 0's chains can start ASAP
            t = hstp.tile([128, KT, 512], BF, tag="hs", name=f"hs{n}")
            for k in range(KT):
                if wload:
                    nc.sync.dma_start(w_sb["q"][:, k, :], w_r["q"][:, k, :])
                nc.sync.dma_start(t[:, k, :], hsT_r[:, k, n * 512:(n + 1) * 512])
                if wload:
                    for p in "kv":
                        nc.sync.dma_start(w_sb[p][:, k, :], w_r[p][:, k, :])
                    if k < SQC:
                        cs = slice(k * 512, (k + 1) * 512)
                        nc.sync.dma_start(cos_sb[:, cs], cos_d[:, cs])
                        nc.sync.dma_start(sin_sb[:, cs], sin_d[:, cs])
            hs_tiles[n] = t

        # deferred PE work (v-transposes, attention epilogues) flushed after
        # the next bulk of independent PE instructions has been emitted
        deferred = []

        def flush():
            for fn in deferred:
                fn()
            deferred.clear()

        # ---------------- projection chunk ----------------
        def emit_chunk(n):
            tok = slice(n * 512, (n + 1) * 512)
            pos = slice((n % SQC) * 512, (n % SQC + 1) * 512)
            hs_t = hs_tiles[n]
            for ci, (p, mi) in enumerate(
                [("q", 0), ("k", 0), ("v", 0), ("q", 1), ("k", 1), ("v", 1)]
            ):
                ps = prjp.tile([128, 512], F32, tag="prj", name=f"pj{n}{ci}")
                for k in range(KT):
                    nc.tensor.matmul(
                        ps,
                        w_sb[p][:, k, mi * 128:(mi + 1) * 128],
                        hs_t[:, k, :],
                        start=(k == 0),
                        stop=(k == KT - 1) and not with_bias,
                    )
                if with_bias:
                    nc.tensor.matmul(
                        ps,
                        b_sb[p][:, mi * 128:(mi + 1) * 128],
                        ones_row,
                        start=False,
                        stop=True,
                    )
                flush()
                if p == "v":
                    # psum -> bf16 sbuf, then (deferred) PE-transpose into
                    # natural [token, d] layout
                    vt = vtp.tile([128, 512], BF, tag="vt")
                    nc.vector.tensor_copy(vt, ps)

                    def vtrans(vt=vt, mi=mi, n=n):
                        tp = stp.tile([128, 2, 512], F32, tag="st", name=f"vt{n}{mi}")
                        for blk in range(4):
                            dst8 = tp[:, 0, blk * 64:(blk + 1) * 64].bitcast(BF)
                            nc.tensor.transpose(
                                dst8, vt[:, blk * 128:(blk + 1) * 128], identB
                            )
                            nc.vector.tensor_copy(vN[mi][:, n * 4 + blk, :], dst8)

                    deferred.append(vtrans)
                else:
                    dst = qT if p == "q" else kTt
                    # RoPE: t1 = ps*cos (gpsimd), t2 = rot(ps)*sin (DVE),
                    # out = t1 + t2 -> bf16 persist (DVE)
                    t1 = ropet.tile([128, 512], F32, tag="t1")
                    nc.vector.tensor_tensor(t1, ps, cos_sb[:, pos], op=ALU.mult)
                    t2 = ropet.tile([128, 512], F32, tag="t2")
                    nc.vector.tensor_tensor(
                        t2[0:64], ps[64:128], sin_sb[64:128, pos], op=ALU.mult
                    )
                    nc.vector.tensor_tensor(
                        t2[64:128], ps[0:64], sin_sb[0:64, pos], op=ALU.mult
                    )
                    nc.vector.tensor_tensor(dst[mi][:, tok], t1, t2, op=ALU.add)

        # ---------------- attention iteration ----------------
        def emit_att(m, b, c):
            sq = slice(b * S + c * 512, b * S + (c + 1) * 512)
            e_t = epool.tile([128, SKT, 512], BF, tag="e", name=f"e{m}{b}{c}")
            ot = otp.tile([128, 512], F32, tag="ot", name=f"ot{m}{b}{c}")
            esA = esump.tile([128, 4, 512], BF, tag="es", name=f"esA{m}{b}{c}")
            esB = esump.tile([128, 4, 512], BF, tag="es", name=f"esB{m}{b}{c}")

            for t in range(8):
                st = stp.tile([128, 2, 512], F32, tag="st", name=f"st{m}{b}{c}{t}")
                for half in range(2):
                    sk = 2 * t + half
                    kblk = kTt[m][:, b * S + sk * 128: b * S + (sk + 1) * 128]
                    nc.tensor.matmul(st[:, half, :], kblk, qT[m][:, sq],
                                     start=True, stop=True)
                if t == 1:
                    flush()
                nc.scalar.activation(
                    e_t[:, 2 * t:2 * t + 2, :], st, AF.Exp,
                    bias=ebias_sb, scale=ISCALE,
                )
                # PV matmuls for the previous tile's pair (let exp run ahead)
                if t >= 1:
                    for sk in (2 * t - 2, 2 * t - 1):
                        nc.tensor.matmul(
                            ot, vN[m][:, b * SKT + sk, :], e_t[:, sk, :],
                            start=(sk == 0), stop=False,
                        )
                if t == 3:
                    nc.vector.tensor_tensor(
                        esA, e_t[:, 0:4, :], e_t[:, 4:8, :], op=ALU.add
                    )
            for sk in (SKT - 2, SKT - 1):
                nc.tensor.matmul(
                    ot, vN[m][:, b * SKT + sk, :], e_t[:, sk, :],
                    start=False, stop=(sk == SKT - 1),
                )
            nc.vector.tensor_tensor(
                esB, e_t[:, 8:12, :], e_t[:, 12:16, :], op=ALU.add
            )
            t2 = es2p.tile([128, 4, 512], BF, tag="es2", name=f"t2{m}{b}{c}")
            nc.vector.tensor_tensor(t2, esA, esB, op=ALU.add)

            def epilogue(m=m, b=b, c=c, ot=ot, t2=t2):
                # denominators with queries on partitions: dn_t[q, qb] so the
                # reciprocal runs 128-wide; deferred into the next iteration
                # so the DVE half-sum latency stays off the PE critical path
                dn_tile = stp.tile([128, 2, 512], F32, tag="st", name=f"dn{m}{b}{c}")
                dn_t = dn_tile[:, 0, 0:4]
                for qb in range(4):
                    qs = slice(qb * 128, (qb + 1) * 128)
                    for j in range(4):
                        nc.tensor.matmul(
                            dn_t[:, qb:qb + 1], t2[:, j, qs], onesK,
                            start=(j == 0), stop=(j == 3),
                        )
                rec_sb = recp.tile([128, 4], F32, tag="rec", name=f"rc{m}{b}{c}")
                nc.vector.reciprocal(rec_sb, dn_t)
                rp = stp.tile([128, 2, 512], F32, tag="st", name=f"rp{m}{b}{c}")
                for qb in range(4):
                    nc.tensor.transpose(
                        rp[0:1, 0, qb * 128:(qb + 1) * 128],
                        rec_sb[:, qb:qb + 1], identF,
                    )
                rec_row = recp.tile([1, 512], F32, tag="recr", name=f"rr{m}{b}{c}")
                nc.vector.tensor_copy(rec_row, rp[0:1, 0, :])
                rec_bc = recp.tile([128, 512], F32, tag="recb", name=f"rb{m}{b}{c}")
                nc.gpsimd.partition_broadcast(rec_bc, rec_row)
                o_sb = opool.tile([128, 512], F32, tag="o", name=f"o{m}{b}{c}")
                nc.vector.tensor_tensor(o_sb, ot, rec_bc, op=ALU.mult)
                nc.sync.dma_start(
                    out_d[m * 128:(m + 1) * 128, b * S + c * 512: b * S + (c + 1) * 512],
                    o_sb,
                )

            deferred.append(epilogue)

        # ---------------- fused schedule ----------------
        att_iters = {
            b: [(m, c) for c in range(SQC) for m in range(HPC)] for b in range(B)
        }
        dma_hs(0, wload=True)
        for n in range(NCH):
            if n + 1 < NCH:
                dma_hs(n + 1)
            emit_chunk(n)
            if n >= SQC:
                # interleave two batch-0 attention iterations per late chunk
                for m, c in att_iters[0][2 * (n - SQC): 2 * (n - SQC) + 2]:
                    emit_att(m, 0, c)
        for m, c in att_iters[0][2 * (NCH - 1 - SQC) + 2:]:
            emit_att(m, 0, c)
        for m, c in att_iters[1]:
            emit_att(m, 1, c)
        flush()

    nc.compile()
    return nc


def _rope_tables():
    inv_freq = 1.0 / (ROPE_BASE ** (np.arange(0, D, 2, dtype=np.float64) / D))
    pos = np.arange(S, dtype=np.float64)
    ang = pos[:, None] * inv_freq[None, :]          # [S, D/2]
    emb = np.concatenate([ang, ang], axis=-1)       # [S, D]
    cosT = np.ascontiguousarray(np.cos(emb).T.astype(np.float32))  # [D, S]
    sinT = np.sin(emb).T.astype(np.float32)
    # swapped + sign-folded so each RoPE half-op reads aligned partitions:
    # t2[0:64] = q[64:128] * sinSw[64:128] (= -sin[0:64])
    # t2[64:128] = q[0:64] * sinSw[0:64]   (= +sin[64:128])
    sinSw = np.concatenate([sinT[64:128], -sinT[0:64]], axis=0)
    return cosT, np.ascontiguousarray(sinSw)


def kernel(hidden_states, Wq, bq, Wk, bk, Wv, bv):
    global LAST_RESULT
    import ml_dtypes
    from concourse.bass_utils import run_bass_kernel_spmd

    BF = ml_dtypes.bfloat16

    hs = np.asarray(hidden_states, dtype=np.float32).reshape(BS, H)
    Wq = np.asarray(Wq, dtype=np.float32)
    Wk = np.asarray(Wk, dtype=np.float32)
    Wv = np.asarray(Wv, dtype=np.float32)
    bq = np.asarray(bq, dtype=np.float32)
    bk = np.asarray(bk, dtype=np.float32)
    bv = np.asarray(bv, dtype=np.float32)

    with_bias = bool(np.any(bq) or np.any(bk) or np.any(bv))
    nc = _build_nc(with_bias)

    hsT = np.ascontiguousarray(hs.T).astype(BF)     # [H, BS]
    cosT, sinT = _rope_tables()

    in_maps = []
    for c in range(NCORES):
        ch = slice(c * CH, (c + 1) * CH)
        m = {
            "hsT": hsT,
            "wqT": np.ascontiguousarray(Wq[ch, :].T).astype(BF),
            "wkT": np.ascontiguousarray(Wk[ch, :].T).astype(BF),
            "wvT": np.ascontiguousarray(Wv[ch, :].T).astype(BF),
            "cosT": cosT,
            "sinT": sinT,
        }
        if with_bias:
            m["bq"] = np.ascontiguousarray(bq[None, ch]).astype(BF)
            m["bk"] = np.ascontiguousarray(bk[None, ch]).astype(BF)
            m["bv"] = np.ascontiguousarray(bv[None, ch]).astype(BF)
        else:
            z = np.zeros((1, CH), dtype=BF)
            m["bq"] = m["bk"] = m["bv"] = z
        in_maps.append(m)

    res = run_bass_kernel_spmd(nc, in_maps, core_ids=list(range(NCORES)))
    LAST_RESULT = res

    full = np.concatenate([r["out"] for r in res.results], axis=0)  # [H, BS]
    return np.ascontiguousarray(full.T).reshape(B, S, H)


# revision 25
# speedup vs baseline: 1.3242x; 1.0139x over previous
"""Multi-head attention (QKV proj + RoPE + softmax attention) on 8 Trainium2
NeuronCores, tensor-parallel over heads (2 heads per core).

v3: bf16 matmul operands, fused projection/attention emission, softmax
denominator via bf16 DVE half-tree + thin [1,512] matmuls (no 32-wide
denominator matmuls), transposed DRAM output ([CH, BS]) with a
broadcast-matmul + GpSimd epilogue (no output/denominator PE transposes).

Contract: kernel(**inputs) takes the FULL unsharded inputs (numpy/jax arrays,
shapes hardcoded below) and returns the FULL [B, S, H] output.
"""

from contextlib import ExitStack

import numpy as np

B, S, H = 2, 2048, 2048
NH, D = 16, 128
ROPE_BASE = 10000.0
NCORES = 8
HPC = NH // NCORES          # heads per core
CH = HPC * D                # output channels per core
BS = B * S                  # flattened tokens
KT = H // 128               # contraction k-tiles
NCH = BS // 512             # 512-wide token chunks
SKT = S // 128              # score k-tiles per sequence
SQC = S // 512              # query chunks per sequence
EBIAS = 2.0                 # exp(score - EBIAS): cancels in softmax division

LAST_RESULT = None          # BassKernelResults of the most recent run (for test.py)


def _build_nc(with_bias):
    import concourse.mybir as mybir
    import concourse.tile as tile
    from concourse import bacc
    from concourse.masks import make_identity

    F32 = mybir.dt.float32
    F32R = mybir.dt.float32r
    BF = mybir.dt.bfloat16
    AF = mybir.ActivationFunctionType
    ALU = mybir.AluOpType
    ISCALE = float(1.0 / np.sqrt(D))

    nc = bacc.Bacc("TRN2", debug=False, enable_partition_id=False)

    hsT_d = nc.dram_tensor("hsT", [H, BS], BF, kind="ExternalInput").ap()
    wT_d = {
        p: nc.dram_tensor(f"w{p}T", [H, CH], BF, kind="ExternalInput").ap()
        for p in "qkv"
    }
    b_d = {
        p: nc.dram_tensor(f"b{p}", [1, CH], BF, kind="ExternalInput").ap()
        for p in "qkv"
    }
    cos_d = nc.dram_tensor("cosT", [D, S], F32, kind="ExternalInput").ap()
    sin_d = nc.dram_tensor("sinT", [D, S], F32, kind="ExternalInput").ap()
    out_d = nc.dram_tensor("out", [CH, BS], F32, kind="ExternalOutput").ap()

    with tile.TileContext(nc) as tc, ExitStack() as ctx:
        # ---- persistent state ----
        persist = ctx.enter_context(tc.tile_pool(name="persist", bufs=1))
        qT = [persist.tile([128, BS], BF, tag=f"qT{m}", name=f"qT{m}") for m in range(HPC)]
        kTt = [persist.tile([128, BS], BF, tag=f"kT{m}", name=f"kT{m}") for m in range(HPC)]
        vN = [persist.tile([128, BS // 128, D], BF, tag=f"v{m}", name=f"vn{m}") for m in range(HPC)]

        consts = ctx.enter_context(tc.tile_pool(name="consts", bufs=1))
        identB = consts.tile([128, 128], BF, tag="identB")
        make_identity(nc, identB)
        identF = consts.tile([128, 128], F32, tag="identF")
        make_identity(nc, identF)
        onesK = consts.tile([128, 1], BF, tag="onesK")
        nc.vector.memset(onesK, 1.0)
        ebias_sb = consts.tile([128, 1], F32, tag="ebias")
        nc.vector.memset(ebias_sb, -EBIAS)
        if with_bias:
            ones_row = consts.tile([1, 512], BF, tag="ones_row")
            nc.vector.memset(ones_row, 1.0)
            b_sb = {}
            for p in "qkv":
                b_sb[p] = consts.tile([1, CH], BF, tag=f"b{p}", name=f"b{p}sb")
                nc.sync.dma_start(b_sb[p], b_d[p])

        # ---- pools (all live for the whole fused kernel) ----
        wpool = ctx.enter_context(tc.tile_pool(name="wpool", bufs=1))
        tabs = ctx.enter_context(tc.tile_pool(name="tabs", bufs=1))
        hstp = ctx.enter_context(tc.tile_pool(name="hstp", bufs=2))
        ropet = ctx.enter_context(tc.tile_pool(name="ropet", bufs=2))
        vtp = ctx.enter_context(tc.tile_pool(name="vtp", bufs=2))
        epool = ctx.enter_context(tc.tile_pool(name="epool", bufs=2))
        esump = ctx.enter_context(tc.tile_pool(name="esump", bufs=4))
        es2p = ctx.enter_context(tc.tile_pool(name="es2p", bufs=2))
        recp = ctx.enter_context(tc.tile_pool(name="recp", bufs=2))
        opool = ctx.enter_context(tc.tile_pool(name="opool", bufs=2))

        prjp = ctx.enter_context(tc.tile_pool(name="prjp", bufs=2, space="PSUM"))
        stp = ctx.enter_context(tc.tile_pool(name="stp", bufs=2, space="PSUM"))
        otp = ctx.enter_context(tc.tile_pool(name="otp", bufs=2, space="PSUM"))

        # ---- table + weight loads ----
        cos_sb = tabs.tile([D, S], F32, tag="cos")
        sin_sb = tabs.tile([D, S], F32, tag="sin")
        w_sb = {}
        for p in "qkv":
            w_sb[p] = wpool.tile([128, KT, CH], BF, tag=f"w{p}", name=f"w{p}sb")
        w_r = {p: wT_d[p].rearrange("(k p) c -> p k c", p=128) for p in "qkv"}

        hsT_r = hsT_d.rearrange("(k p) t -> p k t", p=128)
        hs_tiles = {}

        def dma_hs(n, wload=False):
            # per-k transfers spread across DMA queues (a single dma_start
            # serializes ~2MB onto one ~20GB/s queue); DMAs are emitted in
            # first-use order so chunk 0's chains can start ASAP: chain (q,0)
            # needs wq[k] + hs0[k] ascending, then wk/wv, then the RoPE tables
            t = hstp.tile([128, KT, 512], BF, tag="hs", name=f"hs{n}")
            for k in range(KT):
                if wload:
                    nc.sync.dma_start(w_sb["q"][:, k, :], w_r["q"][:, k, :])
                nc.sync.dma_start(t[:, k, :], hsT_r[:, k, n * 512:(n + 1) * 512])
            if wload:
                for k in range(KT):
                    nc.sync.dma_start(w_sb["k"][:, k, :], w_r["k"][:, k, :])
                    nc.sync.dma_start(w_sb["v"][:, k, :], w_r["v"][:, k, :])
                    if k < SQC:
                        cs = slice(k * 512, (k + 1) * 512)
                        nc.sync.dma_start(cos_sb[:, cs], cos_d[:, cs])
                        nc.sync.dma_start(sin_sb[:, cs], sin_d[:, cs])
            hs_tiles[n] = t

        # deferred PE work (v-transposes, attention epilogues) flushed after
        # the next bulk of independent PE instructions has been emitted
        deferred = []

        def flush():
            for fn in deferred:
                fn()
            deferred.clear()

        # ---------------- projection chunk ----------------
        def emit_chunk(n):
            tok = slice(n * 512, (n + 1) * 512)
            pos = slice((n % SQC) * 512, (n % SQC + 1) * 512)
            hs_t = hs_tiles[n]
            for ci, (p, mi) in enumerate(
                [("q", 0), ("k", 0), ("v", 0), ("q", 1), ("k", 1), ("v", 1)]
            ):
                ps = prjp.tile([128, 512], F32, tag="prj", name=f"pj{n}{ci}")
                for k in range(KT):
                    nc.tensor.matmul(
                        ps,
                        w_sb[p][:, k, mi * 128:(mi + 1) * 128],
                        hs_t[:, k, :],
                        start=(k == 0),
                        stop=(k == KT - 1) and not with_bias,
                    )
                if with_bias:
                    nc.tensor.matmul(
                        ps,
                        b_sb[p][:, mi * 128:(mi + 1) * 128],
                        ones_row,
                        start=False,
                        stop=True,
                    )
                flush()
                if p == "v":
                    # psum -> bf16 sbuf, then (deferred) PE-transpose into
                    # natural [token, d] layout
                    vt = vtp.tile([128, 512], BF, tag="vt")
                    nc.vector.tensor_copy(vt, ps)

                    def vtrans(vt=vt, mi=mi, n=n):
                        tp = stp.tile([128, 2, 512], F32, tag="st", name=f"vt{n}{mi}")
                        for blk in range(4):
                            dst8 = tp[:, 0, blk * 64:(blk + 1) * 64].bitcast(BF)
                            nc.tensor.transpose(
                                dst8, vt[:, blk * 128:(blk + 1) * 128], identB
                            )
                            nc.vector.tensor_copy(vN[mi][:, n * 4 + blk, :], dst8)

                    deferred.append(vtrans)
                else:
                    dst = qT if p == "q" else kTt
                    # RoPE: t1 = ps*cos (gpsimd), t2 = rot(ps)*sin (DVE),
                    # out = t1 + t2 -> bf16 persist (DVE)
                    t1 = ropet.tile([128, 512], F32, tag="t1")
                    nc.vector.tensor_tensor(t1, ps, cos_sb[:, pos], op=ALU.mult)
                    t2 = ropet.tile([128, 512], F32, tag="t2")
                    nc.vector.tensor_tensor(
                        t2[0:64], ps[64:128], sin_sb[64:128, pos], op=ALU.mult
                    )
                    nc.vector.tensor_tensor(
                        t2[64:128], ps[0:64], sin_sb[0:64, pos], op=ALU.mult
                    )
                    nc.vector.tensor_tensor(dst[mi][:, tok], t1, t2, op=ALU.add)

        # ---------------- attention iteration ----------------
        def emit_att(m, b, c):
            sq = slice(b * S + c * 512, b * S + (c + 1) * 512)
            e_t = epool.tile([128, SKT, 512], BF, tag="e", name=f"e{m}{b}{c}")
            ot = otp.tile([128, 512], F32, tag="ot", name=f"ot{m}{b}{c}")
            esA = esump.tile([128, 4, 512], BF, tag="es", name=f"esA{m}{b}{c}")
            esB = esump.tile([128, 4, 512], BF, tag="es", name=f"esB{m}{b}{c}")

            for t in range(8):
                st = stp.tile([128, 2, 512], F32, tag="st", name=f"st{m}{b}{c}{t}")
                for half in range(2):
                    sk = 2 * t + half
                    kblk = kTt[m][:, b * S + sk * 128: b * S + (sk + 1) * 128]
                    nc.tensor.matmul(st[:, half, :], kblk, qT[m][:, sq],
                                     start=True, stop=True)
                if t == 1:
                    flush()
                nc.scalar.activation(
                    e_t[:, 2 * t:2 * t + 2, :], st, AF.Exp,
                    bias=ebias_sb, scale=ISCALE,
                )
                # PV matmuls for the previous tile's pair (let exp run ahead)
                if t >= 1:
                    for sk in (2 * t - 2, 2 * t - 1):
                        nc.tensor.matmul(
                            ot, vN[m][:, b * SKT + sk, :], e_t[:, sk, :],
                            start=(sk == 0), stop=False,
                        )
                if t == 3:
                    nc.vector.tensor_tensor(
                        esA, e_t[:, 0:4, :], e_t[:, 4:8, :], op=ALU.add
                    )
            for sk in (SKT - 2, SKT - 1):
                nc.tensor.matmul(
                    ot, vN[m][:, b * SKT + sk, :], e_t[:, sk, :],
                    start=False, stop=(sk == SKT - 1),
                )
            nc.vector.tensor_tensor(
                esB, e_t[:, 8:12, :], e_t[:, 12:16, :], op=ALU.add
            )
            t2 = es2p.tile([128, 4, 512], BF, tag="es2", name=f"t2{m}{b}{c}")
            nc.vector.tensor_tensor(t2, esA, esB, op=ALU.add)

            def epilogue(m=m, b=b, c=c, ot=ot, t2=t2):
                # denominators with queries on partitions: dn_t[q, qb] so the
                # reciprocal runs 128-wide; deferred into the next iteration
                # so the DVE half-sum latency stays off the PE critical path
                dn_tile = stp.tile([128, 2, 512], F32, tag="st", name=f"dn{m}{b}{c}")
                dn_t = dn_tile[:, 0, 0:4]
                for qb in range(4):
                    qs = slice(qb * 128, (qb + 1) * 128)
                    for j in range(4):
                        nc.tensor.matmul(
                            dn_t[:, qb:qb + 1], t2[:, j, qs], onesK,
                            start=(j == 0), stop=(j == 3),
                        )
                rec_sb = recp.tile([128, 4], F32, tag="rec", name=f"rc{m}{b}{c}")
                nc.vector.reciprocal(rec_sb, dn_t)
                rp = stp.tile([128, 2, 512], F32, tag="st", name=f"rp{m}{b}{c}")
                for qb in range(4):
                    nc.tensor.transpose(
                        rp[0:1, 0, qb * 128:(qb + 1) * 128],
                        rec_sb[:, qb:qb + 1], identF,
                    )
                rec_row = recp.tile([1, 512], F32, tag="recr", name=f"rr{m}{b}{c}")
                nc.vector.tensor_copy(rec_row, rp[0:1, 0, :])
                rec_bc = recp.tile([128, 512], F32, tag="recb", name=f"rb{m}{b}{c}")
                nc.gpsimd.partition_broadcast(rec_bc, rec_row)
                o_sb = opool.tile([128, 512], F32, tag="o", name=f"o{m}{b}{c}")
                nc.vector.tensor_tensor(o_sb, ot, rec_bc, op=ALU.mult)
                nc.sync.dma_start(
                    out_d[m * 128:(m + 1) * 128, b * S + c * 512: b * S + (c + 1) * 512],
                    o_sb,
                )

            deferred.append(epilogue)

        # ---------------- fused schedule ----------------
        att_iters = {
            b: [(m, c) for c in range(SQC) for m in range(HPC)] for b in range(B)
        }
        dma_hs(0, wload=True)
        for n in range(NCH):
            if n + 1 < NCH:
                dma_hs(n + 1)
            emit_chunk(n)
            if n >= SQC:
                # interleave two batch-0 attention iterations per late chunk
                for m, c in att_iters[0][2 * (n - SQC): 2 * (n - SQC) + 2]:
                    emit_att(m, 0, c)
        for m, c in att_iters[0][2 * (NCH - 1 - SQC) + 2:]:
            emit_att(m, 0, c)
        for m, c in att_iters[1]:
            emit_att(m, 1, c)
        flush()

    nc.compile()
    return nc


def _rope_tables():
    inv_freq = 1.0 / (ROPE_BASE ** (np.arange(0, D, 2, dtype=np.float64) / D))
    pos = np.arange(S, dtype=np.float64)
    ang = pos[:, None] * inv_freq[None, :]          # [S, D/2]
    emb = np.concatenate([ang, ang], axis=-1)       # [S, D]
    cosT = np.ascontiguousarray(np.cos(emb).T.astype(np.float32))  # [D, S]
    sinT = np.sin(emb).T.astype(np.float32)
    # swapped + sign-folded so each RoPE half-op reads aligned partitions:
    # t2[0:64] = q[64:128] * sinSw[64:128] (= -sin[0:64])
    # t2[64:128] = q[0:64] * sinSw[0:64]   (= +sin[64:128])
    sinSw = np.concatenate([sinT[64:128], -sinT[0:64]], axis=0)
    return cosT, np.ascontiguousarray(sinSw)


def kernel(hidden_states, Wq, bq, Wk, bk, Wv, bv):
    global LAST_RESULT
    import ml_dtypes
    from concourse.bass_utils import run_bass_kernel_spmd

    BF = ml_dtypes.bfloat16

    hs = np.asarray(hidden_states, dtype=np.float32).reshape(BS, H)
    Wq = np.asarray(Wq, dtype=np.float32)
    Wk = np.asarray(Wk, dtype=np.float32)
    Wv = np.asarray(Wv, dtype=np.float32)
    bq = np.asarray(bq, dtype=np.float32)
    bk = np.asarray(bk, dtype=np.float32)
    bv = np.asarray(bv, dtype=np.float32)

    with_bias = bool(np.any(bq) or np.any(bk) or np.any(bv))
    nc = _build_nc(with_bias)

    hsT = np.ascontiguousarray(hs.T).astype(BF)     # [H, BS]
    cosT, sinT = _rope_tables()

    in_maps = []
    for c in range(NCORES):
        ch = slice(c * CH, (c + 1) * CH)
        m = {
            "hsT": hsT,
            "wqT": np.ascontiguousarray(Wq[ch, :].T).astype(BF),
            "wkT": np.ascontiguousarray(Wk[ch, :].T).astype(BF),
            "wvT": np.ascontiguousarray(Wv[ch, :].T).astype(BF),
            "cosT": cosT,
            "sinT": sinT,
        }
        if with_bias:
            m["bq"] = np.ascontiguousarray(bq[None, ch]).astype(BF)
            m["bk"] = np.ascontiguousarray(bk[None, ch]).astype(BF)
            m["bv"] = np.ascontiguousarray(bv[None, ch]).astype(BF)
        else:
            z = np.zeros((1, CH), dtype=BF)
            m["bq"] = m["bk"] = m["bv"] = z
        in_maps.append(m)

    res = run_bass_kernel_spmd(nc, in_maps, core_ids=list(range(NCORES)))
    LAST_RESULT = res

    full = np.concatenate([r["out"] for r in res.results], axis=0)  # [H, BS]
    return np.ascontiguousarray(full.T).reshape(B, S, H)
